# revision 1
# baseline (speedup 1.0000x reference)
"""Trainium2 Bass kernel for DynamicDirectionalConv.

Math (per batch b):
  x_low = einsum('chw,mc->mhw', x, w_reduce)                 # 1x1 reduce C=256->16
  w_h   = cos(angle)^2
  out_low = w_h * (x_low (*) BASE_H) + (1-w_h) * (x_low (*) BASE_V)
  out   = einsum('mhw,cm->chw', out_low, w_expand)           # 1x1 expand 16->256

The per-pixel blend factors out of the tap sum (weights multiply at the
output pixel), and both base kernels are axis-aligned anisotropic
Gaussians -> separable rank-1 7x7 convs with reflect padding.

Sharding: data-parallel over batch, 1 batch per NeuronCore (B=8, 8 cores).

v2: full bf16 pipeline. x and out travel as bf16 (halves HBM traffic);
output staging interleaves h-row pairs so every DMA descriptor moves a
512B contiguous run (full DMA bandwidth). The cos^2 blend map is applied
pre-replicated from the host like the angle premap before it. The
PSUM->SBUF drain of the expand stage (the one f32 volume that cannot be
16-bit) is split across DVE/Act/Pool.

Layout pipeline (per core):
  x bf16 [c, h, w] --PE (x tiles stationary, wrT moving)--> X3 [w, (h_pad, m)]
  H-pass: symmetric-tap pair sums + FMA chains (DVE + gpsimd)
  W-pass: banded reflect matrices via PE (lhsT = Tw.T, stationary)
  blend with host-provided cos^2 map replicated over m
  pack-transpose on PE -> [(hl8, m), (k, w)], h = 8k + hl
  expand: K=64 bf16 matmuls with zero-padded weight variants
  drain: pso f32 -> ost bf16 with (k, j2, w) interleave, DMA h-row pairs
"""

import math

import numpy as np

import concourse.bass as bass
import concourse.tile as tile
from concourse import mybir
import bass_rust
from concourse.bass_utils import run_bass_kernel_spmd

B, C, H, W, MID = 8, 256, 128, 128, 16
K, PAD = 7, 3
F32 = mybir.dt.float32
BF16 = mybir.dt.bfloat16

ALL_STAGES = frozenset(
    ["indma", "s1", "conv", "wpass", "blend", "pack", "s4", "outdma"]
)


# ----------------------------------------------------------------- host consts
def _host_consts():
    ax = np.linspace(-(K // 2), K // 2, K, dtype=np.float64)
    e_w = np.exp(-(ax**2) / (2 * 2.5**2))  # wide gaussian (sigma_h)
    e_n = np.exp(-(ax**2) / (2 * 1.0**2))  # narrow gaussian (sigma_v)
    # BASE_H[i,j] = e_w[i]*e_n[j]/(S+eps); BASE_V[i,j] = e_n[i]*e_w[j]/(S+eps)
    s_h = float((np.outer(e_w, e_n)).sum()) + 1e-8
    s_v = float((np.outer(e_n, e_w)).sum()) + 1e-8
    gh_A = e_w.astype(np.float32)
    gh_B = e_n.astype(np.float32)
    gw_A = e_n / s_h
    gw_B = e_w / s_v

    def refl(t):
        if t < 0:
            return -t
        if t > W - 1:
            return 2 * (W - 1) - t
        return t

    def banded(g):
        T = np.zeros((W, W), dtype=np.float64)
        for wo in range(W):
            for j in range(K):
                T[wo, refl(wo + j - PAD)] += g[j]
        return T

    TwA = banded(gw_A)  # out = TwA @ Y  (w-conv with reflect)
    TwB = banded(gw_B)
    ident = np.eye(128, dtype=np.float32)
    return gh_A, gh_B, np.ascontiguousarray(TwA.T), np.ascontiguousarray(TwB.T), ident


GH_A, GH_B, TWTA, TWTB, IDENT = _host_consts()


# ----------------------------------------------------------------- bass module
def build_nc(split_multiwaits=True, stages=ALL_STAGES):
    st = frozenset(stages)
    nc = bass.Bass()

    x_in = nc.dram_tensor("x", [C, H, W], BF16, kind="ExternalInput")
    wh_in = nc.dram_tensor("whrep", [128, H * MID], BF16, kind="ExternalInput")
    wrT0_in = nc.dram_tensor("wrT0", [128, MID], BF16, kind="ExternalInput")
    wrT1_in = nc.dram_tensor("wrT1", [128, MID], BF16, kind="ExternalInput")
    twtA_in = nc.dram_tensor("TwTA", [128, 128], BF16, kind="ExternalInput")
    twtB_in = nc.dram_tensor("TwTB", [128, 128], BF16, kind="ExternalInput")
    ident_in = nc.dram_tensor("ident", [128, 128], BF16, kind="ExternalInput")
    wet_in = nc.dram_tensor("WETrep", [128, 4 * C], BF16, kind="ExternalInput")
    out_dram = nc.dram_tensor("out", [C, H, W], BF16, kind="ExternalOutput")

    HP = H + 2 * PAD  # 134 padded rows

    from contextlib import ExitStack

    with tile.TileContext(nc) as tc, ExitStack() as es:
        consts = es.enter_context(tc.tile_pool(name="consts", bufs=1))
        xpool = es.enter_context(tc.tile_pool(name="xpool", bufs=4))
        x3pool = es.enter_context(tc.tile_pool(name="x3", bufs=1))
        ypool = es.enter_context(tc.tile_pool(name="y", bufs=2))
        zpool = es.enter_context(tc.tile_pool(name="z", bufs=2))
        bpool = es.enter_context(tc.tile_pool(name="blend", bufs=2))
        olppool = es.enter_context(tc.tile_pool(name="olp", bufs=2))
        opool = es.enter_context(tc.tile_pool(name="ostage", bufs=2))
        ps1pool = es.enter_context(tc.tile_pool(name="ps1", bufs=1, space="PSUM"))
        pswpool = es.enter_context(tc.tile_pool(name="psw", bufs=2, space="PSUM"))
        pstpool = es.enter_context(tc.tile_pool(name="pst", bufs=1, space="PSUM"))
        psopool = es.enter_context(tc.tile_pool(name="pso", bufs=2, space="PSUM"))

        # ---- constants to SBUF (once)
        wrT0 = consts.tile([128, MID], BF16)
        wrT1 = consts.tile([128, MID], BF16)
        twtA = consts.tile([128, 128], BF16)
        twtB = consts.tile([128, 128], BF16)
        ident = consts.tile([128, 128], BF16)
        wet = consts.tile([128, 4 * C], BF16)
        whrep = consts.tile([128, H * MID], BF16)  # [w, (h, m)]
        nc.sync.dma_start(out=wrT0, in_=wrT0_in[:])
        nc.sync.dma_start(out=wrT1, in_=wrT1_in[:])
        nc.sync.dma_start(out=twtA, in_=twtA_in[:])
        nc.sync.dma_start(out=twtB, in_=twtB_in[:])
        nc.sync.dma_start(out=ident, in_=ident_in[:])
        nc.sync.dma_start(out=wet, in_=wet_in[:])
        nc.sync.dma_start(out=whrep, in_=wh_in[:])

        X3 = x3pool.tile([128, HP * MID], BF16)  # [w, (hp, m)]
        X3r = X3.rearrange("p (hp m) -> p hp m", m=MID)

        # all input DMAs issued up front at 16-row granularity with the two
        # c-halves interleaved, so s1 work unblocks as early as possible and
        # nothing sits ahead of them in the SP queue
        xts = []
        for hg in range(4):
            h0 = hg * 32
            subs = []
            for sub in range(2):
                hh = h0 + sub * 16
                xt0 = xpool.tile([128, 16, W], BF16, tag=f"xt0{sub}")
                xt1 = xpool.tile([128, 16, W], BF16, tag=f"xt1{sub}")
                if "indma" in st:
                    nc.sync.dma_start(out=xt0, in_=x_in[0:128, hh:hh + 16, :])
                    nc.sync.dma_start(out=xt1, in_=x_in[128:256, hh:hh + 16, :])
                subs.append((xt0, xt1))
            xts.append(subs)

        def emit_s1_group(hg):
            """x_low for 32 h rows -> X3 interior rows."""
            h0 = hg * 32
            if "s1" in st:
                ps1 = ps1pool.tile([128, 512], F32, tag="ps1")
                for hl in range(32):
                    fo = hl * MID
                    xt0, xt1 = xts[hg][hl // 16]
                    nc.tensor.matmul(
                        ps1[:, fo:fo + MID], lhsT=xt0[:, hl % 16, :],
                        rhs=wrT0, start=True, stop=False,
                    )
                    nc.tensor.matmul(
                        ps1[:, fo:fo + MID], lhsT=xt1[:, hl % 16, :],
                        rhs=wrT1, start=False, stop=True,
                    )
                nc.scalar.copy(
                    out=X3[:, (PAD + h0) * MID:(PAD + h0 + 32) * MID],
                    in_=ps1,
                )
                if hg == 0:
                    # top reflect: hp 0,1,2 <- hp 6,5,4  (h -k <- h k)
                    for k in range(1, PAD + 1):
                        nc.scalar.copy(
                            out=X3r[:, PAD - k, :], in_=X3r[:, PAD + k, :]
                        )
                if hg == 3:
                    # bottom reflect: h 127+k <- h 127-k
                    for k in range(1, PAD + 1):
                        nc.scalar.copy(
                            out=X3r[:, PAD + H - 1 + k, :],
                            in_=X3r[:, PAD + H - 1 - k, :],
                        )

        ost_tiles = {}
        olps = {}
        ys = {}

        def emit_conv(ch):
            """7-tap H-conv (pair sums + FMA trees) for 32 output rows."""
            h0 = ch * 32

            def xsl(i):
                return X3[:, (h0 + i) * MID:(h0 + i) * MID + 512]

            Ya = ypool.tile([128, 512], BF16, tag="ya")
            Yb = ypool.tile([128, 512], BF16, tag="yb")
            ys[ch] = (Ya, Yb)
            if "conv" in st:
                # symmetric taps: shared pair sums; balanced FMA trees,
                # A on DVE, B's FMAs on gpsimd so the two run concurrently
                # pair sums on gpsimd (TensorTensor is its only fast op)
                s_tiles = []
                for i in range(3):
                    s = ypool.tile([128, 512], BF16, tag=f"s{i}")
                    nc.gpsimd.tensor_add(out=s, in0=xsl(i), in1=xsl(K - 1 - i))
                    s_tiles.append(s)
                # Ya: balanced FMA tree fully on DVE
                ma = ypool.tile([128, 512], BF16, tag="ma")
                va = ypool.tile([128, 512], BF16, tag="va")
                nc.vector.tensor_scalar_mul(ma, xsl(3), float(GH_A[3]))
                nc.vector.tensor_scalar_mul(va, s_tiles[1], float(GH_A[1]))
                nc.vector.scalar_tensor_tensor(
                    out=ma, in0=s_tiles[0], scalar=float(GH_A[0]),
                    in1=ma, op0=mybir.AluOpType.mult, op1=mybir.AluOpType.add,
                )
                nc.vector.scalar_tensor_tensor(
                    out=va, in0=s_tiles[2], scalar=float(GH_A[2]),
                    in1=va, op0=mybir.AluOpType.mult, op1=mybir.AluOpType.add,
                )
                nc.vector.tensor_add(out=Ya, in0=ma, in1=va)
                # Yb: scalar muls on DVE (4x mode), add tree on gpsimd
                pb = []
                for i, (src, g) in enumerate(
                    [(xsl(3), GH_B[3]), (s_tiles[0], GH_B[0]),
                     (s_tiles[1], GH_B[1]), (s_tiles[2], GH_B[2])]
                ):
                    t = ypool.tile([128, 512], BF16, tag=f"pb{i}")
                    nc.vector.tensor_scalar_mul(t, src, float(g))
                    pb.append(t)
                ub = ypool.tile([128, 512], BF16, tag="ub")
                vb = ypool.tile([128, 512], BF16, tag="vb")
                nc.gpsimd.tensor_add(out=ub, in0=pb[0], in1=pb[1])
                nc.gpsimd.tensor_add(out=vb, in0=pb[2], in1=pb[3])
                nc.gpsimd.tensor_add(out=Yb, in0=ub, in1=vb)

        def emit_wpb(ch):
            """wpass + blend + pack for 32 output rows."""
            h0 = ch * 32
            Ya, Yb = ys[ch]
            # W-pass: psa = TwA @ Ya, psb = TwB @ Yb; only psb staged to
            # SBUF (walrus allows at most one PSUM input per instruction)
            psa = pswpool.tile([128, 512], F32, tag="psw")
            psb = pswpool.tile([128, 512], F32, tag="psw")
            Zb = zpool.tile([128, 512], BF16, tag="zb")
            if "wpass" in st:
                nc.tensor.matmul(psa, lhsT=twtA, rhs=Ya, start=True, stop=True)
                nc.tensor.matmul(psb, lhsT=twtB, rhs=Yb, start=True, stop=True)
                nc.scalar.copy(out=Zb, in_=psb)
            # blend: OL = Zb + whrep*(psa - Zb); sub on DVE (PSUM read),
            # mul+add on gpsimd (SBUF-only TensorTensor)
            OL = bpool.tile([128, 512], BF16, tag="ol")
            if "blend" in st:
                d = bpool.tile([128, 512], BF16, tag="d")
                nc.vector.tensor_sub(out=d, in0=psa, in1=Zb)
                p = bpool.tile([128, 512], BF16, tag="p")
                nc.vector.tensor_mul(
                    out=p, in0=d, in1=whrep[:, h0 * MID:h0 * MID + 512]
                )
                nc.vector.tensor_add(out=OL, in0=p, in1=Zb)
            # pack-transpose: [w, (8h,16m)] blocks -> [(8h,16m), w]
            OLp = olppool.tile([128, 512], BF16, tag="olp")
            olps[ch] = OLp
            if "pack" in st:
                pst = pstpool.tile([128, 512], BF16, tag="pst")
                for kb in range(4):
                    nc.tensor.transpose(
                        pst[:, kb * 128:(kb + 1) * 128],
                        OL[:, kb * 128:(kb + 1) * 128],
                        ident,
                    )
                nc.scalar.copy(out=OLp, in_=pst)

        # drain engine per (hlp, cc) slot: gpsimd cannot read PSUM, so the
        # expand drain is split DVE/Act only (Act-heavy: Act op is cheaper)
        DRAIN_ENG = "dadaadaa"

        def emit_s4(ch):
            """expand matmuls + PSUM drain + paired-row store DMAs."""
            OLp = olps[ch]
            for hlp in range(4):
                for cc in range(2):
                    ost = opool.tile(
                        [128, 4, 2, W], BF16, tag=f"ost{hlp}_{cc}"
                    )
                    if "s4" in st:
                        # two expand matmuls (h rows 8k+2*hlp, 8k+2*hlp+1)
                        # into one 2-bank PSUM tile, drained by ONE copy
                        pso = psopool.tile([128, 2, 4, W], F32, tag="pso")
                        for j2 in range(2):
                            hl = hlp * 2 + j2
                            b = 64 * (hl // 4)
                            v = hl % 4
                            nc.tensor.matmul(
                                pso[:, j2],
                                lhsT=wet[b:b + 64,
                                         (v * 2 + cc) * 128:
                                         (v * 2 + cc + 1) * 128],
                                rhs=OLp[b:b + 64, :],
                                start=True, stop=True,
                                tile_position=(b, 0),
                            )
                        # drain PSUM f32 -> bf16 with (k, j2, w) interleave
                        dst = ost.rearrange("c k j w -> c j k w")
                        if DRAIN_ENG[hlp * 2 + cc] == "d":
                            nc.vector.tensor_copy(out=dst, in_=pso)
                        else:
                            nc.scalar.copy(out=dst, in_=pso)
                    if "outdma" in st:
                        # rows h = 8k + hlp*2 + j2, k in [4ch, 4ch+4)
                        o_r = out_dram[cc * 128:(cc + 1) * 128].rearrange(
                            "c (k j) w -> c k j w", j=8
                        )
                        nc.sync.dma_start(
                            out=o_r[:, 4 * ch:4 * ch + 4,
                                    hlp * 2:hlp * 2 + 2, :],
                            in_=ost,
                        )

        # software pipeline: conv of chunk k+1 is emitted before blend/pack
        # of chunk k, and expands before work gated on later input DMAs, so
        # no engine queue head-blocks on a not-yet-ready earlier stage
        emit_s1_group(0)
        emit_s1_group(1)
        emit_conv(0)
        emit_s1_group(2)
        emit_conv(1)
        emit_wpb(0)
        emit_s4(0)
        emit_s1_group(3)
        emit_conv(2)
        emit_wpb(1)
        emit_s4(1)
        emit_conv(3)
        emit_wpb(2)
        emit_s4(2)
        emit_wpb(3)
        emit_s4(3)

    if split_multiwaits:
        _split_multiwaits(nc)
    return nc


def _split_multiwaits(nc):
    """Walrus in this toolchain accepts at most one sync-wait per
    instruction; hoist extras onto same-engine nops just before it."""
    n_new = 0
    for f in nc.m.functions:
        for bb in f.blocks:
            out, changed = [], False
            for ins in bb.instructions:
                si = ins.sync_info
                if si is not None and len(si.on_wait) > 1:
                    changed = True
                    waits = list(si.on_wait)
                    for w in waits[:-1]:
                        n_new += 1
                        nop = bass_rust.InstNoOp(
                            name=f"I-mwsplit-{n_new}", engine=ins.engine
                        )
                        nop.sync_info = mybir.SyncInfo(on_wait=[w], on_update=[])
                        out.append(nop)
                    ins.sync_info = mybir.SyncInfo(
                        on_wait=[waits[-1]], on_update=list(si.on_update)
                    )
                out.append(ins)
            if changed:
                bb.instructions = out
    return n_new


_NC = None


def _get_nc():
    global _NC
    if _NC is None:
        _NC = build_nc()
    return _NC


def make_in_maps(x, angle_map, w_reduce, w_expand):
    bf = mybir.dt.np(BF16)
    wrT = np.ascontiguousarray(w_reduce.T.astype(np.float32))  # [C, MID]
    # wet_ext[p, v*C + c] = w_expand[c, p%16] if (p//16)%4 == v else 0
    wet_rep = np.zeros((128, 4 * C), np.float32)
    weT = w_expand.T.astype(np.float32)  # [MID, C]
    for p in range(128):
        v = (p // 16) % 4
        wet_rep[p, v * C:(v + 1) * C] = weT[p % 16]
    wet_rep = np.ascontiguousarray(wet_rep).astype(bf)
    consts = {
        "wrT0": wrT[0:128].astype(bf),
        "wrT1": wrT[128:256].astype(bf),
        "TwTA": TWTA.astype(bf),
        "TwTB": TWTB.astype(bf),
        "ident": IDENT.astype(bf),
        "WETrep": wet_rep,
    }
    # per-pixel blend map, pre-transposed and replicated over m on host
    # (elementwise prep, same class as the angle premap it replaces)
    wh = np.cos(angle_map.astype(np.float64)) ** 2  # (B, H, W)
    whrep = np.repeat(
        wh.transpose(0, 2, 1)[:, :, :, None], MID, axis=3
    ).reshape(B, 128, H * MID).astype(bf)
    return [
        {
            "x": np.ascontiguousarray(x[i]).astype(bf),
            "whrep": np.ascontiguousarray(whrep[i]),
            **consts,
        }
        for i in range(B)
    ]


def kernel(x, angle_map, w_reduce, w_expand):
    nc = _get_nc()
    in_maps = make_in_maps(x, angle_map, w_reduce, w_expand)
    res = run_bass_kernel_spmd(nc, in_maps, core_ids=list(range(B)))
    return np.stack([r["out"] for r in res.results]).astype(np.float32)



# revision 3
# speedup vs baseline: 1.1200x; 1.1200x over previous
"""Trainium2 Bass kernel for DynamicDirectionalConv (v4).

Math (per batch b):
  x_low = einsum('chw,mc->mhw', x, w_reduce)                 # 1x1 reduce C=256->16
  w_h   = cos(angle)^2
  out_low = w_h * (x_low (*) BASE_H) + (1-w_h) * (x_low (*) BASE_V)
  out   = einsum('mhw,cm->chw', out_low, w_expand)           # 1x1 expand 16->256

Both base kernels are separable rank-1 7x7 gaussians with reflect
padding, and the per-pixel blend factors out of the tap sum.

v4 structural changes vs the v2/v3 baseline:
  * The 7-tap H-conv is FOLDED INTO the reduce matmuls: for input row
    HH, one matmul per c-half streams rhs [c, 2path, 7tap x 16mid]
    (tap coeff x w_reduce pre-multiplied on host) and accumulates into
    a PSUM tile P[w, path, (h' m)] covering a 32-row output chunk.
    The (tap, mid) block lands contiguously on (h', m) columns, so the
    whole H-conv costs only wider matmul streams -- the DVE/gpsimd
    tap FMA pipeline of the old design disappears entirely.
  * PSUM zero-init per chunk via a contraction-1 matmul with a zero
    rhs row (start=True), so accumulating matmuls never clobber.
  * The pack-transpose is FOLDED INTO the W-pass: psaT = Ya.T @ TwT
    with Ya as the stationary operand directly yields the
    [(h8,m16), w] layout the expand stage wants. No separate
    transpose matmuls, no extra PSUM drain.
  * Blend runs in the transposed layout ([(h m), w]) against a
    host-prepared whT map.
  * DMA batching: 2 const DMAs + 16 input tiles + 2 output DMAs per
    chunk (was ~7 + 16 + 32), input issued first so compute starts
    ~4us in.

Sharding: data-parallel over batch, 1 batch per NeuronCore (B=8).
"""

import math

import numpy as np

import concourse.bass as bass
import concourse.tile as tile
from concourse import mybir
import bass_rust
from concourse.bass_utils import run_bass_kernel_spmd

B, C, H, W, MID = 8, 256, 128, 128, 16
K, PAD = 7, 3
F32 = mybir.dt.float32
BF16 = mybir.dt.bfloat16

NCH = 32  # output rows per chunk
NCHUNK = H // NCH


# ----------------------------------------------------------------- host consts
def _refl(t, n):
    if t < 0:
        return -t
    if t > n - 1:
        return 2 * (n - 1) - t
    return t


def _banded_reflect(g, n):
    """T[out, in]: out[o] = sum_t g[t] * x[refl(o + t - PAD)]."""
    T = np.zeros((n, n), dtype=np.float64)
    for o in range(n):
        for t in range(K):
            T[o, _refl(o + t - PAD, n)] += g[t]
    return T


def _host_consts():
    ax = np.linspace(-(K // 2), K // 2, K, dtype=np.float64)
    e_w = np.exp(-(ax**2) / (2 * 2.5**2))  # wide gaussian (sigma_h)
    e_n = np.exp(-(ax**2) / (2 * 1.0**2))  # narrow gaussian (sigma_v)
    s_h = float(np.outer(e_w, e_n).sum()) + 1e-8
    s_v = float(np.outer(e_n, e_w).sum()) + 1e-8
    gh = [e_w, e_n]  # h-axis taps for paths A, B (unnormalized)
    gw = [e_n / s_h, e_w / s_v]  # w-axis taps carry the normalization

    ThA = _banded_reflect(gh[0], H)  # [h_out, h_in]
    ThB = _banded_reflect(gh[1], H)
    # interior columns must equal the pure tap pattern (g symmetric)
    for Th, g in ((ThA, gh[0]), (ThB, gh[1])):
        for HH in range(4, H - 4):
            col = Th[HH - PAD:HH + PAD + 1, HH]
            assert np.allclose(col, g[::-1]) and np.allclose(col, g)

    TwTA = np.ascontiguousarray(_banded_reflect(gw[0], W).T)  # [w_in, w_out]
    TwTB = np.ascontiguousarray(_banded_reflect(gw[1], W).T)
    return gh, (ThA, ThB), (TwTA, TwTB)


GH, TH, TWT = _host_consts()

# const-tile column layout (all [128, cols] bf16)
OFF_TWTA = 0
OFF_TWTB = 128
OFF_WET = 256
OFF_WHT = OFF_WET + 4 * C          # 1280
NC_LATE = OFF_WHT + H * MID        # 3328

OFF_RINT = 0                       # 2ch x (2path x 112)
OFF_RTOP = OFF_RINT + 2 * 224      # 448: 4r x 2ch x 224
OFF_RBOT = OFF_RTOP + 8 * 224      # 2240
OFF_ZERO = OFF_RBOT + 8 * 224      # 4032
NC_EARLY = OFF_ZERO + 512          # 4544


def _build_const_early(w_reduce):
    """RINT / RTOP / RBOT coefficient blocks + zero region."""
    wrT = w_reduce.T.astype(np.float64)  # [C, MID]
    ce = np.zeros((128, NC_EARLY), dtype=np.float64)

    def block(ch, coefs):
        # [128c, 2path, 7tap*16m] for one c-half; coefs[path][j] scales tap j
        blk = np.zeros((128, 2, K * MID), dtype=np.float64)
        for p in range(2):
            for j in range(K):
                blk[:, p, j * MID:(j + 1) * MID] = (
                    coefs[p][j] * wrT[ch * 128:(ch + 1) * 128]
                )
        return blk.reshape(128, 2 * K * MID)

    for ch in range(2):
        # interior: out h' = HH-3+j gets tap coeff g[HH - h' + 3] = g[6-j]
        ce[:, OFF_RINT + ch * 224: OFF_RINT + (ch + 1) * 224] = block(
            ch, [g[::-1] for g in GH]
        )
    for r in range(4):  # top edge rows HH = r, window h' = 0..6
        coefs = [Th[0:K, r] for Th in TH]
        for ch in range(2):
            o = OFF_RTOP + (r * 2 + ch) * 224
            ce[:, o:o + 224] = block(ch, coefs)
    for r in range(4):  # bottom edge rows HH = 124 + r, window h' = 121..127
        coefs = [Th[H - K:H, 124 + r] for Th in TH]
        for ch in range(2):
            o = OFF_RBOT + (r * 2 + ch) * 224
            ce[:, o:o + 224] = block(ch, coefs)
    return ce


def _build_const_late(w_expand, angle_map_b):
    cl = np.zeros((128, NC_LATE), dtype=np.float64)
    cl[:, OFF_TWTA:OFF_TWTA + 128] = TWT[0]
    cl[:, OFF_TWTB:OFF_TWTB + 128] = TWT[1]
    # wet[p, v*C + c] = w_expand[c, p%16] if (p//16)%4 == v else 0
    weT = w_expand.T.astype(np.float64)  # [MID, C]
    wet = np.zeros((128, 4 * C), np.float64)
    for p in range(128):
        v = (p // 16) % 4
        wet[p, v * C:(v + 1) * C] = weT[p % 16]
    cl[:, OFF_WET:OFF_WET + 4 * C] = wet
    # whT[p=(hl,m), kb16, w] = cos(angle[h=8*kb+hl, w])^2
    wh = np.cos(angle_map_b.astype(np.float64)) ** 2  # [H, W]
    hl = (np.arange(128) // MID)  # [p]
    kb = np.arange(H // 8)  # [16]
    whT = wh[(8 * kb[None, :] + hl[:, None])]  # [p, 16, W]
    cl[:, OFF_WHT:OFF_WHT + H * MID] = whT.reshape(128, H * MID)
    return cl


def _chunk_rows(k):
    """(HH, kind) contributors for chunk k; kind: 0=interior, 1=top, 2=bot."""
    rows = []
    if k == 0:
        rows += [(r, 1) for r in range(4)]
    lo = max(4, NCH * k - PAD)
    hi = min(H - 5, NCH * k + NCH - 1 + PAD)
    rows += [(r, 0) for r in range(lo, hi + 1)]
    if k == NCHUNK - 1:
        rows += [(r, 2) for r in range(H - 4, H)]
    return rows


# ----------------------------------------------------------------- bass module
def build_nc(split_multiwaits=True):
    nc = bass.Bass()

    x_in = nc.dram_tensor("x", [C, H, W], BF16, kind="ExternalInput")
    ce_in = nc.dram_tensor("c_early", [128, NC_EARLY], BF16, kind="ExternalInput")
    cl_in = nc.dram_tensor("c_late", [128, NC_LATE], BF16, kind="ExternalInput")
    out_dram = nc.dram_tensor("out", [C, H, W], BF16, kind="ExternalOutput")

    from contextlib import ExitStack

    with tile.TileContext(nc) as tc, ExitStack() as es:
        consts = es.enter_context(tc.tile_pool(name="consts", bufs=1))
        xpool = es.enter_context(tc.tile_pool(name="xpool", bufs=1))
        ypool = es.enter_context(tc.tile_pool(name="y", bufs=2))
        zpool = es.enter_context(tc.tile_pool(name="z", bufs=2))
        olppool = es.enter_context(tc.tile_pool(name="olp", bufs=2))
        opool = es.enter_context(tc.tile_pool(name="ostage", bufs=2))
        ppool = es.enter_context(tc.tile_pool(name="P", bufs=1, space="PSUM"))
        pswpool = es.enter_context(tc.tile_pool(name="psw", bufs=1, space="PSUM"))
        psopool = es.enter_context(tc.tile_pool(name="pso", bufs=2, space="PSUM"))

        cearly = consts.tile([128, NC_EARLY], BF16)
        nc.sync.dma_start(out=cearly, in_=ce_in[:])

        # input tiles: 16 h-rows x c-half each, first three h-groups up
        # front so chunk 0 can start ~4us in; late consts next; rest after
        xt = [[None, None] for _ in range(8)]

        def dma_x(g):
            for ch in range(2):
                t = xpool.tile([128, 16, W], BF16, tag=f"x{g}_{ch}")
                nc.sync.dma_start(
                    out=t, in_=x_in[ch * 128:(ch + 1) * 128, g * 16:(g + 1) * 16, :]
                )
                xt[g][ch] = t

        for g in range(3):
            dma_x(g)
        clate = consts.tile([128, NC_LATE], BF16)
        nc.sync.dma_start(out=clate, in_=cl_in[:])
        for g in range(3, 8):
            dma_x(g)

        # const views
        RINT = [
            cearly[:, OFF_RINT + ch * 224:OFF_RINT + (ch + 1) * 224].rearrange(
                "c (p j) -> c p j", p=2
            )
            for ch in range(2)
        ]
        RTOP = [
            [
                cearly[:, OFF_RTOP + (r * 2 + ch) * 224:
                       OFF_RTOP + (r * 2 + ch + 1) * 224].rearrange(
                    "c (p j) -> c p j", p=2
                )
                for ch in range(2)
            ]
            for r in range(4)
        ]
        RBOT = [
            [
                cearly[:, OFF_RBOT + (r * 2 + ch) * 224:
                       OFF_RBOT + (r * 2 + ch + 1) * 224].rearrange(
                    "c (p j) -> c p j", p=2
                )
                for ch in range(2)
            ]
            for r in range(4)
        ]
        ZROW = cearly[0:1, OFF_ZERO:OFF_ZERO + 512]
        ZCOL = cearly[0:1, OFF_ZERO:OFF_ZERO + 128]
        TWTAv = clate[:, OFF_TWTA:OFF_TWTA + 128]
        TWTBv = clate[:, OFF_TWTB:OFF_TWTB + 128]
        WETv = clate[:, OFF_WET:OFF_WET + 4 * C]
        WHTv = clate[:, OFF_WHT:OFF_WHT + H * MID].rearrange(
            "p (kb w) -> p kb w", kb=H // 8
        )

        Ps = {}
        Ys = {}
        psws = {}
        olps = {}

        def emit_s1h(k):
            """reduce + H-conv for 32 output rows into P[w, path, (h' m)]."""
            P = ppool.tile([128, 2, NCH * MID], F32, tag="P")
            Ps[k] = P
            nc.tensor.matmul(
                P[:, 0], lhsT=ZCOL, rhs=ZROW,
                start=True, stop=False, skip_group_check=True,
            )
            nc.tensor.matmul(
                P[:, 1], lhsT=ZCOL, rhs=ZROW,
                start=True, stop=False, skip_group_check=True,
            )
            rows = _chunk_rows(k)
            for i, (HH, kind) in enumerate(rows):
                for ch in range(2):
                    lhsT = xt[HH // 16][ch][:, HH % 16, :]
                    if kind == 1:  # top edge, window h' = 0..6 (chunk 0)
                        rhs = RTOP[HH][ch]
                        outw = P[:, :, 0:K * MID]
                    elif kind == 2:  # bottom edge, window h' = 121..127
                        rhs = RBOT[HH - (H - 4)][ch]
                        outw = P[:, :, (NCH - K) * MID:NCH * MID]
                    else:
                        lo = max(HH - PAD, NCH * k)
                        hi = min(HH + PAD, NCH * k + NCH - 1)
                        j0 = lo - (HH - PAD)
                        j1 = hi - (HH - PAD)
                        rhs = RINT[ch][:, :, j0 * MID:(j1 + 1) * MID]
                        outw = P[:, :, (lo - NCH * k) * MID:(hi + 1 - NCH * k) * MID]
                    last = (i == len(rows) - 1) and (ch == 1)
                    nc.tensor.matmul(
                        outw, lhsT=lhsT, rhs=rhs,
                        start=False, stop=last, skip_group_check=True,
                    )

        def emit_drain(k):
            """P -> Ya/Yb SBUF bf16 [w, (h m)]."""
            P = Ps[k]
            Ya = ypool.tile([128, NCH * MID], BF16, tag="ya")
            Yb = ypool.tile([128, NCH * MID], BF16, tag="yb")
            Ys[k] = (Ya, Yb)
            nc.vector.tensor_copy(out=Ya, in_=P[:, 0])
            nc.scalar.copy(out=Yb, in_=P[:, 1])

        def emit_wpt(k):
            """W-conv + transpose fused: psaT[(h m), w'] = Ya.T @ TwT."""
            Ya, Yb = Ys[k]
            psaT = pswpool.tile([128, 4, W], F32, tag="psa")
            psbT = pswpool.tile([128, 4, W], F32, tag="psb")
            psws[k] = (psaT, psbT)
            for kb in range(4):
                nc.tensor.matmul(
                    psaT[:, kb], lhsT=Ya[:, kb * 128:(kb + 1) * 128],
                    rhs=TWTAv, start=True, stop=True,
                )
                nc.tensor.matmul(
                    psbT[:, kb], lhsT=Yb[:, kb * 128:(kb + 1) * 128],
                    rhs=TWTBv, start=True, stop=True,
                )

        def emit_blend(k):
            """OLp = Zb + whT*(psaT - Zb) in [(h m), (kb w)] layout."""
            psaT, psbT = psws[k]
            Zb = zpool.tile([128, 4, W], BF16, tag="zb")
            nc.scalar.copy(out=Zb, in_=psbT)
            d = zpool.tile([128, 4, W], BF16, tag="d")
            nc.vector.tensor_sub(out=d, in0=psaT, in1=Zb)
            p = zpool.tile([128, 4, W], BF16, tag="p")
            nc.vector.tensor_mul(out=p, in0=d, in1=WHTv[:, 4 * k:4 * k + 4, :])
            OLp = olppool.tile([128, 4 * W], BF16, tag="olp")
            OLr = OLp.rearrange("p (kb w) -> p kb w", kb=4)
            nc.gpsimd.tensor_add(out=OLr, in0=p, in1=Zb)
            olps[k] = OLp

        def emit_expand(k):
            """expand 16->256 + drains + 2 output DMAs."""
            OLp = olps[k]
            osts = []
            for cc in range(2):
                ost = opool.tile([128, 4, 8, W], BF16, tag=f"ost{cc}")
                osts.append(ost)
            for hlp in range(4):
                for cc in range(2):
                    pso = psopool.tile([128, 2, 4, W], F32, tag="pso")
                    for j2 in range(2):
                        hl = hlp * 2 + j2
                        b = 64 * (hl // 4)
                        v = hl % 4
                        nc.tensor.matmul(
                            pso[:, j2],
                            lhsT=WETv[b:b + 64,
                                      (v * 2 + cc) * 128:(v * 2 + cc + 1) * 128],
                            rhs=OLp[b:b + 64, :],
                            start=True, stop=True,
                            tile_position=(b, 0),
                        )
                    dst = osts[cc][:, :, hlp * 2:hlp * 2 + 2, :].rearrange(
                        "c k j w -> c j k w"
                    )
                    if (hlp + cc) % 2 == 0:
                        nc.vector.tensor_copy(out=dst, in_=pso)
                    else:
                        nc.scalar.copy(out=dst, in_=pso)
            for cc in range(2):
                o_r = out_dram[cc * 128:(cc + 1) * 128].rearrange(
                    "c (kb hl) w -> c kb hl w", hl=8
                )
                nc.sync.dma_start(
                    out=o_r[:, 4 * k:4 * k + 4, :, :], in_=osts[cc]
                )

        # software pipeline
        emit_s1h(0)
        emit_drain(0)
        emit_s1h(1)
        emit_wpt(0)
        emit_blend(0)
        emit_drain(1)
        emit_expand(0)
        emit_s1h(2)
        emit_wpt(1)
        emit_blend(1)
        emit_drain(2)
        emit_expand(1)
        emit_s1h(3)
        emit_wpt(2)
        emit_blend(2)
        emit_drain(3)
        emit_expand(2)
        emit_wpt(3)
        emit_blend(3)
        emit_expand(3)

    if split_multiwaits:
        _split_multiwaits(nc)
    return nc


def _split_multiwaits(nc):
    """Walrus in this toolchain accepts at most one sync-wait per
    instruction; hoist extras onto same-engine nops just before it."""
    n_new = 0
    for f in nc.m.functions:
        for bb in f.blocks:
            out, changed = [], False
            for ins in bb.instructions:
                si = ins.sync_info
                if si is not None and len(si.on_wait) > 1:
                    changed = True
                    waits = list(si.on_wait)
                    for w in waits[:-1]:
                        n_new += 1
                        nop = bass_rust.InstNoOp(
                            name=f"I-mwsplit-{n_new}", engine=ins.engine
                        )
                        nop.sync_info = mybir.SyncInfo(on_wait=[w], on_update=[])
                        out.append(nop)
                    ins.sync_info = mybir.SyncInfo(
                        on_wait=[waits[-1]], on_update=list(si.on_update)
                    )
                out.append(ins)
            if changed:
                bb.instructions = out
    return n_new


_NC = None


def _get_nc():
    global _NC
    if _NC is None:
        _NC = build_nc()
    return _NC


def make_in_maps(x, angle_map, w_reduce, w_expand):
    bf = mybir.dt.np(BF16)
    ce = _build_const_early(np.asarray(w_reduce, np.float64)).astype(bf)
    cl_shared = _build_const_late(
        np.asarray(w_expand, np.float64), np.zeros((H, W))
    )
    maps = []
    for i in range(B):
        cl = cl_shared.copy()
        wh = np.cos(np.asarray(angle_map[i], np.float64)) ** 2
        hl = np.arange(128) // MID
        kb = np.arange(H // 8)
        whT = wh[(8 * kb[None, :] + hl[:, None])]
        cl[:, OFF_WHT:OFF_WHT + H * MID] = whT.reshape(128, H * MID)
        maps.append(
            {
                "x": np.ascontiguousarray(np.asarray(x[i])).astype(bf),
                "c_early": ce,
                "c_late": cl.astype(bf),
            }
        )
    return maps


def kernel(x, angle_map, w_reduce, w_expand):
    nc = _get_nc()
    in_maps = make_in_maps(x, angle_map, w_reduce, w_expand)
    res = run_bass_kernel_spmd(nc, in_maps, core_ids=list(range(B)))
    return np.stack([r["out"] for r in res.results]).astype(np.float32)


# revision 5
# speedup vs baseline: 1.3012x; 1.1617x over previous
"""Trainium2 Bass kernel for DynamicDirectionalConv (v5).

Math (per batch b):
  x_low = einsum('chw,mc->mhw', x, w_reduce)                 # 1x1 reduce C=256->16
  w_h   = cos(angle)^2
  out_low = w_h * (x_low (*) BASE_H) + (1-w_h) * (x_low (*) BASE_V)
  out   = einsum('mhw,cm->chw', out_low, w_expand)           # 1x1 expand 16->256

Both base kernels are separable rank-1 7x7 gaussians with reflect
padding, and the per-pixel blend factors out of the tap sum.

v5 structure -- every linear stage runs on the PE:
  * s1 reduce: per h-row, two 16-col matmuls (c-halves) with the x
    tile stationary -> X3[w, (h m)] (cheap: ldweights pipelines).
  * W-conv + transpose fused: psW[(h8,m16), w'] = X3_block.T @ TwT
    (banded reflect matrix as the moving operand, X3 block
    stationary). One matmul per 8-row block per path.
  * H-conv as a BLOCK-TRIDIAGONAL matmul in the transposed layout:
    h lives in partitions (8 rows per block), so the 7-tap reflect
    conv over h is out[q] = sum_d L[d].T-style products with
    d in {-1,0,+1}; L matrices are shift-invariant except the first /
    last block (reflect folds). Stationaries are reused across blocks
    (ordered by d), accumulation in PSUM.
  * blend in the [(h m), w] layout against a host-prepared whT map.
  * expand 16->256 with the zero-masked replicated weight trick,
    per-(hlp,cc,j2) PSUM tiles, drains alternating DVE/Act,
    2 output DMAs per chunk (contiguous 2KB runs in DRAM).
  * ~8 zero matmuls at t~7.5us (gated only by a tiny const DMA) keep
    the PE busy during the input-DMA head so it ramps to full p-state
    before real work arrives.

Sharding: data-parallel over batch, 1 batch per NeuronCore (B=8).
"""

import math

import numpy as np

import concourse.bass as bass
import concourse.tile as tile
from concourse import mybir
import bass_rust
from concourse.bass_utils import run_bass_kernel_spmd

B, C, H, W, MID = 8, 256, 128, 128, 16
K, PAD = 7, 3
F32 = mybir.dt.float32
BF16 = mybir.dt.bfloat16

NCH = 32  # output rows per chunk
NCHUNK = H // NCH
NB = H // 8  # 16 8-row blocks


# ----------------------------------------------------------------- host consts
def _refl(t, n):
    if t < 0:
        return -t
    if t > n - 1:
        return 2 * (n - 1) - t
    return t


def _banded_reflect(g, n):
    """T[out, in]: out[o] = sum_t g[t] * x[refl(o + t - PAD)]."""
    T = np.zeros((n, n), dtype=np.float64)
    for o in range(n):
        for t in range(K):
            T[o, _refl(o + t - PAD, n)] += g[t]
    return T


def _host_consts():
    ax = np.linspace(-(K // 2), K // 2, K, dtype=np.float64)
    e_w = np.exp(-(ax**2) / (2 * 2.5**2))  # wide gaussian (sigma_h)
    e_n = np.exp(-(ax**2) / (2 * 1.0**2))  # narrow gaussian (sigma_v)
    s_h = float(np.outer(e_w, e_n).sum()) + 1e-8
    s_v = float(np.outer(e_n, e_w).sum()) + 1e-8
    gh = [e_w, e_n]  # h-axis taps for paths A, B (unnormalized)
    gw = [e_n / s_h, e_w / s_v]  # w-axis taps carry the normalization

    Th = [_banded_reflect(g, H) for g in gh]  # [h_out, h_in] per path
    TwT = [np.ascontiguousarray(_banded_reflect(g, W).T) for g in gw]

    # block-tridiagonal factorization of Th into (delta, variant) L mats:
    # L[p=(hl,m), p'=(hl',m')] = Th[8*q_out+hl', 8*q_in+hl] * (m==m')
    # variants: 0:interior d=-1, 1:interior d=0, 2:interior d=+1,
    #           3:d=0 for block 0 (top folds), 4:d=0 for block 15
    Ls = []
    for Tp in Th:
        mats = []
        for (qo, qi) in ((1, 0), (1, 1), (1, 2), (0, 0), (NB - 1, NB - 1)):
            Lm = np.zeros((128, 128), np.float64)
            for hl in range(8):
                for hlp in range(8):
                    v = Tp[8 * qo + hlp, 8 * qi + hl]
                    if v != 0.0:
                        for m in range(MID):
                            Lm[hl * MID + m, hlp * MID + m] = v
            mats.append(Lm)
        Ls.append(mats)
        # verify: assembled block-tridiag reproduces Th exactly
        full = np.zeros((H, H))
        for qo in range(NB):
            for qi in range(NB):
                d = qi - qo
                if abs(d) > 1:
                    assert np.allclose(Tp[8*qo:8*qo+8, 8*qi:8*qi+8], 0)
                    continue
                if d == 0:
                    Lm = mats[3] if qo == 0 else (mats[4] if qo == NB - 1 else mats[1])
                else:
                    Lm = mats[d + 1]
                blk = np.zeros((8, 8))
                for hl in range(8):
                    for hlp in range(8):
                        blk[hlp, hl] = Lm[hl * MID, hlp * MID]
                full[8*qo:8*qo+8, 8*qi:8*qi+8] = blk
        assert np.allclose(full, Tp), "block-tridiag mismatch"
    return gh, Th, TwT, Ls


GH, TH, TWT, LS = _host_consts()

# const layouts
NC_EARLY = 32 + 512  # wrT halves [128,16]x2 + zero row region

OFF_TWTA = 0
OFF_TWTB = 128
OFF_L = 256                        # 2 paths x 5 variants x 128
OFF_WET = OFF_L + 10 * 128         # 1536
OFF_WHT = OFF_WET + 4 * C          # 2560
NC_LATE = OFF_WHT + H * MID        # 4608


def _build_const_early(w_reduce):
    ce = np.zeros((128, NC_EARLY), dtype=np.float64)
    wrT = w_reduce.T.astype(np.float64)  # [C, MID]
    ce[:, 0:MID] = wrT[0:128]
    ce[:, MID:2 * MID] = wrT[128:256]
    return ce


def _build_const_late(w_expand):
    cl = np.zeros((128, NC_LATE), dtype=np.float64)
    cl[:, OFF_TWTA:OFF_TWTA + 128] = TWT[0]
    cl[:, OFF_TWTB:OFF_TWTB + 128] = TWT[1]
    for p in range(2):
        for v in range(5):
            o = OFF_L + (p * 5 + v) * 128
            cl[:, o:o + 128] = LS[p][v]
    weT = w_expand.T.astype(np.float64)  # [MID, C]
    wet = np.zeros((128, 4 * C), np.float64)
    for p in range(128):
        v = (p // 16) % 4
        wet[p, v * C:(v + 1) * C] = weT[p % 16]
    cl[:, OFF_WET:OFF_WET + 4 * C] = wet
    return cl


def _whT(angle_map_b):
    wh = np.cos(np.asarray(angle_map_b, np.float64)) ** 2  # [H, W]
    hl = np.arange(128) // MID
    kb = np.arange(NB)
    return wh[(8 * kb[None, :] + hl[:, None])].reshape(128, H * MID)


# ----------------------------------------------------------------- bass module
def build_nc(split_multiwaits=True):
    nc = bass.Bass()

    x_in = nc.dram_tensor("x", [C, H, W], BF16, kind="ExternalInput")
    ce_in = nc.dram_tensor("c_early", [128, NC_EARLY], BF16, kind="ExternalInput")
    cl_in = nc.dram_tensor("c_late", [128, NC_LATE], BF16, kind="ExternalInput")
    out_dram = nc.dram_tensor("out", [C, H, W], BF16, kind="ExternalOutput")

    from contextlib import ExitStack

    with tile.TileContext(nc) as tc, ExitStack() as es:
        consts = es.enter_context(tc.tile_pool(name="consts", bufs=1))
        xpool = es.enter_context(tc.tile_pool(name="xpool", bufs=1))
        x3pool = es.enter_context(tc.tile_pool(name="x3", bufs=1))
        uwpool = es.enter_context(tc.tile_pool(name="uw", bufs=1))
        zpool = es.enter_context(tc.tile_pool(name="z", bufs=2))
        olppool = es.enter_context(tc.tile_pool(name="olp", bufs=2))
        opool = es.enter_context(tc.tile_pool(name="ostage", bufs=2))
        ps1pool = es.enter_context(tc.tile_pool(name="ps1", bufs=1, space="PSUM"))
        pswpool = es.enter_context(tc.tile_pool(name="psw", bufs=1, space="PSUM"))
        pshpool = es.enter_context(tc.tile_pool(name="psh", bufs=1, space="PSUM"))
        psopool = es.enter_context(tc.tile_pool(name="pso", bufs=3, space="PSUM"))

        cearly = consts.tile([128, NC_EARLY], BF16)
        nc.sync.dma_start(out=cearly, in_=ce_in[:])

        xt = [[None, None] for _ in range(8)]

        def dma_x(g):
            for ch in range(2):
                t = xpool.tile([128, 16, W], BF16, tag=f"x{g}_{ch}")
                nc.sync.dma_start(
                    out=t, in_=x_in[ch * 128:(ch + 1) * 128, g * 16:(g + 1) * 16, :]
                )
                xt[g][ch] = t

        for g in range(3):
            dma_x(g)
        clate = consts.tile([128, NC_LATE], BF16)
        nc.sync.dma_start(out=clate, in_=cl_in[:])
        for g in range(3, 8):
            dma_x(g)

        WR = [cearly[:, 0:MID], cearly[:, MID:2 * MID]]
        ZROW = cearly[0:1, 32:32 + 512]
        ZCOL = cearly[0:1, 32:32 + 128]
        TWTv = [clate[:, OFF_TWTA:OFF_TWTA + 128],
                clate[:, OFF_TWTB:OFF_TWTB + 128]]
        Lv = [
            [clate[:, OFF_L + (p * 5 + v) * 128:OFF_L + (p * 5 + v + 1) * 128]
             for v in range(5)]
            for p in range(2)
        ]
        WETv = clate[:, OFF_WET:OFF_WET + 4 * C]
        WHTv = clate[:, OFF_WHT:OFF_WHT + H * MID].rearrange(
            "p (kb w) -> p kb w", kb=NB
        )

        X3 = x3pool.tile([128, H * MID], BF16)  # [w, (h m)]
        Uw = [uwpool.tile([128, NB, W], BF16, tag=f"uw{p}", name=f"uw{p}")
              for p in range(2)]  # [(hl m), (blk, w')] per path

        ps1s, pshs, olps = {}, {}, {}

        def emit_warmup(n):
            """dummy matmuls into the ps1 slot: ramp PE during DMA head."""
            for i in range(n):
                ps1 = ps1pool.tile([128, 512], F32, tag="ps1")
                nc.tensor.matmul(
                    ps1, lhsT=ZCOL, rhs=ZROW,
                    start=True, stop=True, skip_group_check=True,
                )

        def emit_s1(k):
            """x_low for 32 h rows -> psum [w, (hl m)]."""
            ps1 = ps1pool.tile([128, 512], F32, tag="ps1")
            ps1s[k] = ps1
            for hl in range(32):
                h = NCH * k + hl
                fo = hl * MID
                nc.tensor.matmul(
                    ps1[:, fo:fo + MID], lhsT=xt[h // 16][0][:, h % 16, :],
                    rhs=WR[0], start=True, stop=False,
                )
                nc.tensor.matmul(
                    ps1[:, fo:fo + MID], lhsT=xt[h // 16][1][:, h % 16, :],
                    rhs=WR[1], start=False, stop=True,
                )

        def emit_s1drain(k):
            eng = nc.vector.tensor_copy if k % 2 == 0 else nc.scalar.copy
            kw = {"out": X3[:, NCH * MID * k:NCH * MID * (k + 1)], "in_": ps1s[k]}
            eng(**kw)

        def emit_wt(k):
            """psW[(hl m), w'] = X3_block.T @ TwT; drain to Uw."""
            psw = pswpool.tile([128, 2, 4, W], F32, tag="psw")
            for p in range(2):
                for kb in range(4):
                    qb = 4 * k + kb
                    nc.tensor.matmul(
                        psw[:, p, kb],
                        lhsT=X3[:, qb * 128:(qb + 1) * 128],
                        rhs=TWTv[p], start=True, stop=True,
                    )
            nc.vector.tensor_copy(
                out=Uw[0][:, 4 * k:4 * k + 4, :], in_=psw[:, 0]
            )
            nc.scalar.copy(out=Uw[1][:, 4 * k:4 * k + 4, :], in_=psw[:, 1])

        def emit_ht(k):
            """block-tridiagonal H-conv: psH[q] = sum_d L[d] blocks."""
            psh = pshpool.tile([128, 2, 4, W], F32, tag="psh")
            pshs[k] = psh
            # PSUM accumulation groups must be consecutive matmuls on HW:
            # emit each (path, block)'s 2-3 taps back to back.
            for p in range(2):
                for kb in range(4):
                    qo = 4 * k + kb
                    ds = [d for d in (-1, 0, 1) if 0 <= qo + d < NB]
                    for i, d in enumerate(ds):
                        qi = qo + d
                        if d == 0:
                            v = 3 if qo == 0 else (4 if qo == NB - 1 else 1)
                        else:
                            v = d + 1
                        nc.tensor.matmul(
                            psh[:, p, kb], lhsT=Lv[p][v],
                            rhs=Uw[p][:, qi, :],
                            start=(i == 0), stop=(i == len(ds) - 1),
                        )

        def emit_blend(k):
            """OLp = Zb + whT*(psH_A - Zb) in [(hl m), (kb w)] layout."""
            psh = pshs[k]
            Zb = zpool.tile([128, 4, W], BF16, tag="zb")
            nc.scalar.copy(out=Zb, in_=psh[:, 1])
            d = zpool.tile([128, 4, W], BF16, tag="d")
            nc.vector.tensor_sub(out=d, in0=psh[:, 0], in1=Zb)
            p = zpool.tile([128, 4, W], BF16, tag="p")
            nc.gpsimd.tensor_mul(out=p, in0=d, in1=WHTv[:, 4 * k:4 * k + 4, :])
            OLp = olppool.tile([128, 4 * W], BF16, tag="olp")
            OLr = OLp.rearrange("p (kb w) -> p kb w", kb=4)
            nc.gpsimd.tensor_add(out=OLr, in0=p, in1=Zb)
            olps[k] = OLp

        def emit_expand(k):
            """expand 16->256 + drains + 2 output DMAs."""
            OLp = olps[k]
            osts = []
            for cc in range(2):
                ost = opool.tile([128, 4, 8, W], BF16, tag=f"ost{cc}")
                osts.append(ost)
            for hlp in range(4):
                for cc in range(2):
                    for j2 in range(2):
                        hl = hlp * 2 + j2
                        b = 64 * (hl // 4)
                        v = hl % 4
                        pso = psopool.tile([128, 4, W], F32, tag="pso")
                        nc.tensor.matmul(
                            pso,
                            lhsT=WETv[b:b + 64,
                                      (v * 2 + cc) * 128:(v * 2 + cc + 1) * 128],
                            rhs=OLp[b:b + 64, :],
                            start=True, stop=True,
                            tile_position=(b, 0),
                        )
                        dst = osts[cc][:, :, hl, :]
                        if (hlp + cc + j2) % 2 == 0:
                            nc.vector.tensor_copy(out=dst, in_=pso)
                        else:
                            nc.scalar.copy(out=dst, in_=pso)
            for cc in range(2):
                o_r = out_dram[cc * 128:(cc + 1) * 128].rearrange(
                    "c (kb hl) w -> c kb hl w", hl=8
                )
                nc.sync.dma_start(
                    out=o_r[:, 4 * k:4 * k + 4, :, :], in_=osts[cc]
                )

        # software pipeline
        emit_warmup(8)
        emit_s1(0)
        emit_s1drain(0)
        emit_s1(1)
        emit_s1drain(1)
        emit_wt(0)
        emit_s1(2)
        emit_s1drain(2)
        emit_wt(1)
        emit_ht(0)
        emit_blend(0)
        emit_s1(3)
        emit_s1drain(3)
        emit_wt(2)
        emit_ht(1)
        emit_blend(1)
        emit_expand(0)
        emit_wt(3)
        emit_ht(2)
        emit_blend(2)
        emit_expand(1)
        emit_ht(3)
        emit_blend(3)
        emit_expand(2)
        emit_expand(3)

    if split_multiwaits:
        _split_multiwaits(nc)
    return nc


def _split_multiwaits(nc):
    """Walrus in this toolchain accepts at most one sync-wait per
    instruction; hoist extras onto same-engine nops just before it."""
    n_new = 0
    for f in nc.m.functions:
        for bb in f.blocks:
            out, changed = [], False
            for ins in bb.instructions:
                si = ins.sync_info
                if si is not None and len(si.on_wait) > 1:
                    changed = True
                    waits = list(si.on_wait)
                    for w in waits[:-1]:
                        n_new += 1
                        nop = bass_rust.InstNoOp(
                            name=f"I-mwsplit-{n_new}", engine=ins.engine
                        )
                        nop.sync_info = mybir.SyncInfo(on_wait=[w], on_update=[])
                        out.append(nop)
                    ins.sync_info = mybir.SyncInfo(
                        on_wait=[waits[-1]], on_update=list(si.on_update)
                    )
                out.append(ins)
            if changed:
                bb.instructions = out
    return n_new


_NC = None


def _get_nc():
    global _NC
    if _NC is None:
        _NC = build_nc()
    return _NC


def make_in_maps(x, angle_map, w_reduce, w_expand):
    bf = mybir.dt.np(BF16)
    ce = _build_const_early(np.asarray(w_reduce, np.float64)).astype(bf)
    cl_shared = _build_const_late(np.asarray(w_expand, np.float64))
    maps = []
    for i in range(B):
        cl = cl_shared.copy()
        cl[:, OFF_WHT:OFF_WHT + H * MID] = _whT(angle_map[i])
        maps.append(
            {
                "x": np.ascontiguousarray(np.asarray(x[i])).astype(bf),
                "c_early": ce,
                "c_late": cl.astype(bf),
            }
        )
    return maps


def kernel(x, angle_map, w_reduce, w_expand):
    nc = _get_nc()
    in_maps = make_in_maps(x, angle_map, w_reduce, w_expand)
    res = run_bass_kernel_spmd(nc, in_maps, core_ids=list(range(B)))
    return np.stack([r["out"] for r in res.results]).astype(np.float32)


# revision 8
# speedup vs baseline: 1.3303x; 1.0224x over previous
"""Trainium2 Bass kernel for DynamicDirectionalConv (v5).

Math (per batch b):
  x_low = einsum('chw,mc->mhw', x, w_reduce)                 # 1x1 reduce C=256->16
  w_h   = cos(angle)^2
  out_low = w_h * (x_low (*) BASE_H) + (1-w_h) * (x_low (*) BASE_V)
  out   = einsum('mhw,cm->chw', out_low, w_expand)           # 1x1 expand 16->256

Both base kernels are separable rank-1 7x7 gaussians with reflect
padding, and the per-pixel blend factors out of the tap sum.

v5 structure -- every linear stage runs on the PE:
  * s1 reduce: per h-row, two 16-col matmuls (c-halves) with the x
    tile stationary -> X3[w, (h m)] (cheap: ldweights pipelines).
  * W-conv + transpose fused: psW[(h8,m16), w'] = X3_block.T @ TwT
    (banded reflect matrix as the moving operand, X3 block
    stationary). One matmul per 8-row block per path.
  * H-conv as a BLOCK-TRIDIAGONAL matmul in the transposed layout:
    h lives in partitions (8 rows per block), so the 7-tap reflect
    conv over h is out[q] = sum_d L[d].T-style products with
    d in {-1,0,+1}; L matrices are shift-invariant except the first /
    last block (reflect folds). Stationaries are reused across blocks
    (ordered by d), accumulation in PSUM.
  * blend in the [(h m), w] layout against a host-prepared whT map.
  * expand 16->256 with the zero-masked replicated weight trick,
    per-(hlp,cc,j2) PSUM tiles, drains alternating DVE/Act,
    2 output DMAs per chunk (contiguous 2KB runs in DRAM).
  * ~8 zero matmuls at t~7.5us (gated only by a tiny const DMA) keep
    the PE busy during the input-DMA head so it ramps to full p-state
    before real work arrives.

Sharding: data-parallel over batch, 1 batch per NeuronCore (B=8).
"""

import math

import numpy as np

import concourse.bass as bass
import concourse.tile as tile
from concourse import mybir
import bass_rust
from concourse.bass_utils import run_bass_kernel_spmd

B, C, H, W, MID = 8, 256, 128, 128, 16
K, PAD = 7, 3
F32 = mybir.dt.float32
BF16 = mybir.dt.bfloat16

NCH = 32  # output rows per chunk
NCHUNK = H // NCH
NB = H // 8  # 16 8-row blocks


# ----------------------------------------------------------------- host consts
def _refl(t, n):
    if t < 0:
        return -t
    if t > n - 1:
        return 2 * (n - 1) - t
    return t


def _banded_reflect(g, n):
    """T[out, in]: out[o] = sum_t g[t] * x[refl(o + t - PAD)]."""
    T = np.zeros((n, n), dtype=np.float64)
    for o in range(n):
        for t in range(K):
            T[o, _refl(o + t - PAD, n)] += g[t]
    return T


def _host_consts():
    ax = np.linspace(-(K // 2), K // 2, K, dtype=np.float64)
    e_w = np.exp(-(ax**2) / (2 * 2.5**2))  # wide gaussian (sigma_h)
    e_n = np.exp(-(ax**2) / (2 * 1.0**2))  # narrow gaussian (sigma_v)
    s_h = float(np.outer(e_w, e_n).sum()) + 1e-8
    s_v = float(np.outer(e_n, e_w).sum()) + 1e-8
    gh = [e_w, e_n]  # h-axis taps for paths A, B (unnormalized)
    gw = [e_n / s_h, e_w / s_v]  # w-axis taps carry the normalization

    Th = [_banded_reflect(g, H) for g in gh]  # [h_out, h_in] per path
    TwT = [np.ascontiguousarray(_banded_reflect(g, W).T) for g in gw]

    # block-tridiagonal factorization of Th into (delta, variant) L mats:
    # L[p=(hl,m), p'=(hl',m')] = Th[8*q_out+hl', 8*q_in+hl] * (m==m')
    # variants: 0:interior d=-1, 1:interior d=0, 2:interior d=+1,
    #           3:d=0 for block 0 (top folds), 4:d=0 for block 15
    Ls = []
    for Tp in Th:
        mats = []
        for (qo, qi) in ((1, 0), (1, 1), (1, 2), (0, 0), (NB - 1, NB - 1)):
            Lm = np.zeros((128, 128), np.float64)
            for hl in range(8):
                for hlp in range(8):
                    v = Tp[8 * qo + hlp, 8 * qi + hl]
                    if v != 0.0:
                        for m in range(MID):
                            Lm[hl * MID + m, hlp * MID + m] = v
            mats.append(Lm)
        Ls.append(mats)
        # verify: assembled block-tridiag reproduces Th exactly
        full = np.zeros((H, H))
        for qo in range(NB):
            for qi in range(NB):
                d = qi - qo
                if abs(d) > 1:
                    assert np.allclose(Tp[8*qo:8*qo+8, 8*qi:8*qi+8], 0)
                    continue
                if d == 0:
                    Lm = mats[3] if qo == 0 else (mats[4] if qo == NB - 1 else mats[1])
                else:
                    Lm = mats[d + 1]
                blk = np.zeros((8, 8))
                for hl in range(8):
                    for hlp in range(8):
                        blk[hlp, hl] = Lm[hl * MID, hlp * MID]
                full[8*qo:8*qo+8, 8*qi:8*qi+8] = blk
        assert np.allclose(full, Tp), "block-tridiag mismatch"
    return gh, Th, TwT, Ls


GH, TH, TWT, LS = _host_consts()

# const layouts
NC_EARLY = 32 + 512  # wrT halves [128,16]x2 + zero row region

OFF_TWTA = 0
OFF_TWTB = 128
OFF_L = 256                        # 2 paths x 5 variants x 128
OFF_WET = OFF_L + 10 * 128         # 1536
OFF_WHT = OFF_WET + 4 * C          # 2560
NC_LATE = OFF_WHT + H * MID        # 4608


def _build_const_early(w_reduce):
    ce = np.zeros((128, NC_EARLY), dtype=np.float64)
    wrT = w_reduce.T.astype(np.float64)  # [C, MID]
    ce[:, 0:MID] = wrT[0:128]
    ce[:, MID:2 * MID] = wrT[128:256]
    return ce


def _build_const_late(w_expand):
    cl = np.zeros((128, NC_LATE), dtype=np.float64)
    cl[:, OFF_TWTA:OFF_TWTA + 128] = TWT[0]
    cl[:, OFF_TWTB:OFF_TWTB + 128] = TWT[1]
    for p in range(2):
        for v in range(5):
            o = OFF_L + (p * 5 + v) * 128
            cl[:, o:o + 128] = LS[p][v]
    weT = w_expand.T.astype(np.float64)  # [MID, C]
    wet = np.zeros((128, 4 * C), np.float64)
    for p in range(128):
        v = (p // 16) % 4
        wet[p, v * C:(v + 1) * C] = weT[p % 16]
    cl[:, OFF_WET:OFF_WET + 4 * C] = wet
    return cl


def _whT(angle_map_b):
    wh = np.cos(np.asarray(angle_map_b, np.float64)) ** 2  # [H, W]
    hl = np.arange(128) // MID
    kb = np.arange(NB)
    return wh[(8 * kb[None, :] + hl[:, None])].reshape(128, H * MID)


# ----------------------------------------------------------------- bass module
def build_nc(split_multiwaits=True):
    nc = bass.Bass()

    x_in = nc.dram_tensor("x", [C, H, W], BF16, kind="ExternalInput")
    ce_in = nc.dram_tensor("c_early", [128, NC_EARLY], BF16, kind="ExternalInput")
    cl_in = nc.dram_tensor("c_late", [128, NC_LATE], BF16, kind="ExternalInput")
    out_dram = nc.dram_tensor("out", [C, H, W], BF16, kind="ExternalOutput")

    from contextlib import ExitStack

    with tile.TileContext(nc) as tc, ExitStack() as es:
        consts = es.enter_context(tc.tile_pool(name="consts", bufs=1))
        xpool = es.enter_context(tc.tile_pool(name="xpool", bufs=1))
        x3pool = es.enter_context(tc.tile_pool(name="x3", bufs=1))
        uwpool = es.enter_context(tc.tile_pool(name="uw", bufs=1))
        zpool = es.enter_context(tc.tile_pool(name="z", bufs=2))
        olppool = es.enter_context(tc.tile_pool(name="olp", bufs=2))
        opool = es.enter_context(tc.tile_pool(name="ostage", bufs=2))
        ps1pool = es.enter_context(tc.tile_pool(name="ps1", bufs=1, space="PSUM"))
        pswpool = es.enter_context(tc.tile_pool(name="psw", bufs=1, space="PSUM"))
        pshpool = es.enter_context(tc.tile_pool(name="psh", bufs=1, space="PSUM"))
        psopool = es.enter_context(tc.tile_pool(name="pso", bufs=3, space="PSUM"))

        cearly = consts.tile([128, NC_EARLY], BF16)
        nc.sync.dma_start(out=cearly, in_=ce_in[:])

        xt = [[None, None] for _ in range(8)]

        def dma_x(g):
            for ch in range(2):
                t = xpool.tile([128, 16, W], BF16, tag=f"x{g}_{ch}")
                nc.sync.dma_start(
                    out=t, in_=x_in[ch * 128:(ch + 1) * 128, g * 16:(g + 1) * 16, :]
                )
                xt[g][ch] = t

        # issue order tuned to expected consumption times: conv consts
        # (TwT + L) after the first two x groups, the fat expand/blend
        # consts (wet + whT) only before the first expand needs them
        clate = consts.tile([128, NC_LATE], BF16)
        for g in range(2):
            dma_x(g)
        nc.sync.dma_start(out=clate[:, 0:OFF_WET], in_=cl_in[:, 0:OFF_WET])
        for g in range(2, 6):
            dma_x(g)
        nc.sync.dma_start(out=clate[:, OFF_WET:], in_=cl_in[:, OFF_WET:])
        for g in range(6, 8):
            dma_x(g)

        WR = [cearly[:, 0:MID], cearly[:, MID:2 * MID]]
        ZROW = cearly[0:1, 32:32 + 512]
        ZCOL = cearly[0:1, 32:32 + 128]
        TWTv = [clate[:, OFF_TWTA:OFF_TWTA + 128],
                clate[:, OFF_TWTB:OFF_TWTB + 128]]
        Lv = [
            [clate[:, OFF_L + (p * 5 + v) * 128:OFF_L + (p * 5 + v + 1) * 128]
             for v in range(5)]
            for p in range(2)
        ]
        WETv = clate[:, OFF_WET:OFF_WET + 4 * C]
        WHTv = clate[:, OFF_WHT:OFF_WHT + H * MID].rearrange(
            "p (kb w) -> p kb w", kb=NB
        )

        X3 = x3pool.tile([128, H * MID], BF16)  # [w, (h m)]
        Uw = [uwpool.tile([128, NB, W], BF16, tag=f"uw{p}", name=f"uw{p}")
              for p in range(2)]  # [(hl m), (blk, w')] per path

        ps1s, pshs, olps = {}, {}, {}

        def emit_warmup(n):
            """dummy matmuls into the ps1 slot: ramp PE during DMA head."""
            for i in range(n):
                ps1 = ps1pool.tile([128, 512], F32, tag="ps1")
                nc.tensor.matmul(
                    ps1, lhsT=ZCOL, rhs=ZROW,
                    start=True, stop=True, skip_group_check=True,
                )

        def emit_s1(k):
            """x_low for 32 h rows -> psum [w, (hl m)]."""
            ps1 = ps1pool.tile([128, 512], F32, tag="ps1")
            ps1s[k] = ps1
            for hl in range(32):
                h = NCH * k + hl
                fo = hl * MID
                nc.tensor.matmul(
                    ps1[:, fo:fo + MID], lhsT=xt[h // 16][0][:, h % 16, :],
                    rhs=WR[0], start=True, stop=False,
                )
                nc.tensor.matmul(
                    ps1[:, fo:fo + MID], lhsT=xt[h // 16][1][:, h % 16, :],
                    rhs=WR[1], start=False, stop=True,
                )

        def emit_s1drain(k):
            eng = nc.vector.tensor_copy if k % 2 == 0 else nc.scalar.copy
            kw = {"out": X3[:, NCH * MID * k:NCH * MID * (k + 1)], "in_": ps1s[k]}
            eng(**kw)

        def emit_wt(k):
            """psW[(hl m), w'] = X3_block.T @ TwT; drain to Uw."""
            psw = pswpool.tile([128, 2, 4, W], F32, tag="psw")
            for p in range(2):
                for kb in range(4):
                    qb = 4 * k + kb
                    nc.tensor.matmul(
                        psw[:, p, kb],
                        lhsT=X3[:, qb * 128:(qb + 1) * 128],
                        rhs=TWTv[p], start=True, stop=True,
                    )
            nc.vector.tensor_copy(
                out=Uw[0][:, 4 * k:4 * k + 4, :], in_=psw[:, 0]
            )
            nc.scalar.copy(out=Uw[1][:, 4 * k:4 * k + 4, :], in_=psw[:, 1])

        def emit_ht(k):
            """block-tridiagonal H-conv: psH[q] = sum_d L[d] blocks."""
            psh = pshpool.tile([128, 2, 4, W], F32, tag="psh")
            pshs[k] = psh
            # Wide rhs: one matmul applies the same L block-diagonally to
            # several blocks at once. PSUM accumulation groups (per bank =
            # per path here) are kept consecutive; sequential groups in
            # one bank are fine.
            for p in range(2):
                if k == 0:
                    nc.tensor.matmul(psh[:, p, 0], lhsT=Lv[p][3],
                                     rhs=Uw[p][:, 0, :], start=True, stop=False)
                    nc.tensor.matmul(psh[:, p, 0], lhsT=Lv[p][2],
                                     rhs=Uw[p][:, 1, :], start=False, stop=True)
                    nc.tensor.matmul(psh[:, p, 1:4], lhsT=Lv[p][0],
                                     rhs=Uw[p][:, 0:3, :], start=True, stop=False)
                    nc.tensor.matmul(psh[:, p, 1:4], lhsT=Lv[p][1],
                                     rhs=Uw[p][:, 1:4, :], start=False, stop=False)
                    nc.tensor.matmul(psh[:, p, 1:4], lhsT=Lv[p][2],
                                     rhs=Uw[p][:, 2:5, :], start=False, stop=True)
                elif k == NCHUNK - 1:
                    nc.tensor.matmul(psh[:, p, 0:3], lhsT=Lv[p][0],
                                     rhs=Uw[p][:, 11:14, :], start=True, stop=False)
                    nc.tensor.matmul(psh[:, p, 0:3], lhsT=Lv[p][1],
                                     rhs=Uw[p][:, 12:15, :], start=False, stop=False)
                    nc.tensor.matmul(psh[:, p, 0:3], lhsT=Lv[p][2],
                                     rhs=Uw[p][:, 13:16, :], start=False, stop=True)
                    nc.tensor.matmul(psh[:, p, 3], lhsT=Lv[p][0],
                                     rhs=Uw[p][:, 14, :], start=True, stop=False)
                    nc.tensor.matmul(psh[:, p, 3], lhsT=Lv[p][4],
                                     rhs=Uw[p][:, 15, :], start=False, stop=True)
                else:
                    q0 = 4 * k
                    nc.tensor.matmul(psh[:, p], lhsT=Lv[p][0],
                                     rhs=Uw[p][:, q0 - 1:q0 + 3, :],
                                     start=True, stop=False)
                    nc.tensor.matmul(psh[:, p], lhsT=Lv[p][1],
                                     rhs=Uw[p][:, q0:q0 + 4, :],
                                     start=False, stop=False)
                    nc.tensor.matmul(psh[:, p], lhsT=Lv[p][2],
                                     rhs=Uw[p][:, q0 + 1:q0 + 5, :],
                                     start=False, stop=True)

        def emit_blend(k):
            """OLp = Zb + whT*(psH_A - Zb) in [(hl m), (kb w)] layout."""
            psh = pshs[k]
            Zb = zpool.tile([128, 4, W], BF16, tag="zb")
            nc.scalar.copy(out=Zb, in_=psh[:, 1])
            d = zpool.tile([128, 4, W], BF16, tag="d")
            nc.vector.tensor_sub(out=d, in0=psh[:, 0], in1=Zb)
            p = zpool.tile([128, 4, W], BF16, tag="p")
            nc.gpsimd.tensor_mul(out=p, in0=d, in1=WHTv[:, 4 * k:4 * k + 4, :])
            OLp = olppool.tile([128, 4 * W], BF16, tag="olp")
            OLr = OLp.rearrange("p (kb w) -> p kb w", kb=4)
            nc.gpsimd.tensor_add(out=OLr, in0=p, in1=Zb)
            olps[k] = OLp

        def emit_expand(k):
            """expand 16->256 + drains + 2 output DMAs."""
            OLp = olps[k]
            osts = []
            for cc in range(2):
                ost = opool.tile([128, 4, 8, W], BF16, tag=f"ost{cc}")
                osts.append(ost)
            for hlp in range(4):
                for cc in range(2):
                    for j2 in range(2):
                        hl = hlp * 2 + j2
                        b = 64 * (hl // 4)
                        v = hl % 4
                        pso = psopool.tile([128, 4, W], F32, tag="pso")
                        nc.tensor.matmul(
                            pso,
                            lhsT=WETv[b:b + 64,
                                      (v * 2 + cc) * 128:(v * 2 + cc + 1) * 128],
                            rhs=OLp[b:b + 64, :],
                            start=True, stop=True,
                            tile_position=(b, 0),
                        )
                        dst = osts[cc][:, :, hl, :]
                        if (hlp + cc + j2) % 2 == 0:
                            nc.vector.tensor_copy(out=dst, in_=pso)
                        else:
                            nc.scalar.copy(out=dst, in_=pso)
            for cc in range(2):
                o_r = out_dram[cc * 128:(cc + 1) * 128].rearrange(
                    "c (kb hl) w -> c kb hl w", hl=8
                )
                nc.sync.dma_start(
                    out=o_r[:, 4 * k:4 * k + 4, :, :], in_=osts[cc]
                )

        # software pipeline, ordered by expected operand readiness so the
        # in-order PE queue never head-blocks on a DMA-gated s1 while
        # later compute is ready
        emit_warmup(8)
        emit_s1(0)
        emit_s1drain(0)
        emit_s1(1)
        emit_s1drain(1)
        emit_wt(0)
        emit_s1(2)
        emit_s1drain(2)
        emit_wt(1)
        emit_ht(0)
        emit_blend(0)
        emit_expand(0)
        emit_s1(3)
        emit_s1drain(3)
        emit_wt(2)
        emit_ht(1)
        emit_blend(1)
        emit_expand(1)
        emit_wt(3)
        emit_ht(2)
        emit_blend(2)
        emit_expand(2)
        emit_ht(3)
        emit_blend(3)
        emit_expand(3)

    if split_multiwaits:
        _split_multiwaits(nc)
    return nc


def _split_multiwaits(nc):
    """Walrus in this toolchain accepts at most one sync-wait per
    instruction; hoist extras onto same-engine nops just before it."""
    n_new = 0
    for f in nc.m.functions:
        for bb in f.blocks:
            out, changed = [], False
            for ins in bb.instructions:
                si = ins.sync_info
                if si is not None and len(si.on_wait) > 1:
                    changed = True
                    waits = list(si.on_wait)
                    for w in waits[:-1]:
                        n_new += 1
                        nop = bass_rust.InstNoOp(
                            name=f"I-mwsplit-{n_new}", engine=ins.engine
                        )
                        nop.sync_info = mybir.SyncInfo(on_wait=[w], on_update=[])
                        out.append(nop)
                    ins.sync_info = mybir.SyncInfo(
                        on_wait=[waits[-1]], on_update=list(si.on_update)
                    )
                out.append(ins)
            if changed:
                bb.instructions = out
    return n_new


_NC = None


def _get_nc():
    global _NC
    if _NC is None:
        _NC = build_nc()
    return _NC


def make_in_maps(x, angle_map, w_reduce, w_expand):
    bf = mybir.dt.np(BF16)
    ce = _build_const_early(np.asarray(w_reduce, np.float64)).astype(bf)
    cl_shared = _build_const_late(np.asarray(w_expand, np.float64))
    maps = []
    for i in range(B):
        cl = cl_shared.copy()
        cl[:, OFF_WHT:OFF_WHT + H * MID] = _whT(angle_map[i])
        maps.append(
            {
                "x": np.ascontiguousarray(np.asarray(x[i])).astype(bf),
                "c_early": ce,
                "c_late": cl.astype(bf),
            }
        )
    return maps


def kernel(x, angle_map, w_reduce, w_expand):
    nc = _get_nc()
    in_maps = make_in_maps(x, angle_map, w_reduce, w_expand)
    res = run_bass_kernel_spmd(nc, in_maps, core_ids=list(range(B)))
    return np.stack([r["out"] for r in res.results]).astype(np.float32)
